# revision 4
# baseline (speedup 1.0000x reference)
import os
import sys

import numpy as np

for _p in ("/opt/trn_rl_repo",):
    if _p not in sys.path and os.path.isdir(_p):
        sys.path.append(_p)

N = 1500          # proposals
A = 64            # action classes
NC_CLS = 81       # detection classes
STD = 0.3
PERSON_IDX = 2
SCALE = 1.0 / (STD * STD)          # exp(SCALE * mm)

NCORES = 8
NO = 1536         # padded object count (12 tiles of 128)
P = 128
NT = NO // P      # 12 object tiles per core
HP = 3            # persons per core
BATCH = NCORES * HP                # 24 persons per device batch
KK = 6 * HP + 1   # 19 logical contraction rows
K3 = 3 * KK       # 57 rows after [Ahi; Alo; Ahi] x [Bhi; Bhi; Blo] stacking
NF = HP * A       # 192 output cols per core (person-local x action)
WB = NF + NO      # blob cols: [B | A]
PST = 256         # psum col stride per tile (2 tiles/bank)
ACT_CH = (4, 4, 4)  # activation chunks, in tiles (bank aligned: 4*256*4B = 2 banks)

TCLAMP = 16.0
LNFLOOR = -20000.0


def _hilo(a):
    hi = a.astype(np.float16)
    lo = (a - hi.astype(np.float32)).astype(np.float16)
    return hi, lo


def _prep(bbox, scores, target_mean, action_logits):
    """Host-side: person selection, box geometry, per-core blobs."""
    best = scores.max(axis=1)
    idx = scores.argmax(axis=1)
    person = idx == PERSON_IDX
    hidx = np.where(person)[0]

    w = bbox[:, 2] - bbox[:, 0]
    h = bbox[:, 3] - bbox[:, 1]
    cx = bbox[:, 0] + 0.5 * w
    cy = bbox[:, 1] + 0.5 * h

    cx_o = np.zeros(NO, np.float32); cx_o[:N] = cx
    cy_o = np.zeros(NO, np.float32); cy_o[:N] = cy
    lw_o = np.zeros(NO, np.float32); lw_o[:N] = np.log(w)
    lh_o = np.zeros(NO, np.float32); lh_o[:N] = np.log(h)
    lnrow = np.full(NO, LNFLOOR, np.float32)
    obj = np.where(person, 0.0, best)
    pos = obj > 0
    lnrow[:N] = np.where(
        pos, np.log(np.maximum(obj, 1e-38)) / SCALE, LNFLOOR
    )
    geo = (cx_o, cy_o, lw_o, lh_o, lnrow, w, h, cx, cy)
    return best, hidx, geo


def _batch_blobs(hb, k, geo, target_mean):
    """Build per-core [K3, WB] fp16 blobs for one batch of <=BATCH persons."""
    cx_o, cy_o, lw_o, lh_o, lnrow, w, h, cx, cy = geo

    invw = np.ones(BATCH, np.float32); invw[:k] = 1.0 / w[hb]
    invh = np.ones(BATCH, np.float32); invh[:k] = 1.0 / h[hb]
    cxh = np.zeros(BATCH, np.float32); cxh[:k] = cx[hb]
    cyh = np.zeros(BATCH, np.float32); cyh[:k] = cy[hb]
    lwh = np.zeros(BATCH, np.float32); lwh[:k] = np.log(w[hb])
    lhh = np.zeros(BATCH, np.float32); lhh[:k] = np.log(h[hb])
    mu = np.zeros((BATCH, A, 4), np.float32); mu[:k] = target_mean[hb]
    m2 = (mu * mu).sum(axis=-1)                      # [BATCH, A]

    tx = np.clip(cx_o[None] * invw[:, None] - (cxh * invw)[:, None],
                 -TCLAMP, TCLAMP)
    ty = np.clip(cy_o[None] * invh[:, None] - (cyh * invh)[:, None],
                 -TCLAMP, TCLAMP)
    tw = np.clip(lw_o[None] - lwh[:, None], -TCLAMP, TCLAMP)
    th = np.clip(lh_o[None] - lhh[:, None], -TCLAMP, TCLAMP)
    e2 = tx * tx + ty * ty + tw * tw + th * th

    # object-side rows T[h, r, o], r in {tx,ty,tw,th, -e2/2, 1}
    T = np.empty((BATCH, 6, NO), np.float32)
    T[:, 0] = tx; T[:, 1] = ty; T[:, 2] = tw; T[:, 3] = th
    T[:, 4] = -0.5 * e2
    T[:, 5] = 1.0

    blobs = []
    for c in range(NCORES):
        a32 = np.empty((KK, NO), np.float32)
        a32[:6 * HP] = T[c * HP:(c + 1) * HP].reshape(6 * HP, NO)
        a32[6 * HP] = lnrow

        b32 = np.zeros((KK, NF), np.float32)
        for j in range(HP):
            hh = c * HP + j
            blk = slice(j * A, (j + 1) * A)
            for cc in range(4):
                b32[6 * j + cc, blk] = mu[hh, :, cc]
            b32[6 * j + 4, blk] = 1.0
            b32[6 * j + 5, blk] = -0.5 * m2[hh]
        b32[6 * HP, :] = 1.0

        ahi, alo = _hilo(a32)
        bhi, blo = _hilo(b32)
        blob = np.empty((K3, WB), np.float16)
        blob[0 * KK:1 * KK, :NF] = bhi
        blob[1 * KK:2 * KK, :NF] = bhi
        blob[2 * KK:3 * KK, :NF] = blo
        blob[0 * KK:1 * KK, NF:] = ahi
        blob[1 * KK:2 * KK, NF:] = alo
        blob[2 * KK:3 * KK, NF:] = ahi
        blobs.append(blob)
    return blobs


_NC_CACHE = {}


def _build_nc():
    if "nc" in _NC_CACHE:
        return _NC_CACHE["nc"]
    import concourse.bacc as bacc
    import concourse.mybir as mybir
    from concourse.tile import TileContext

    f32 = mybir.dt.float32
    f16 = mybir.dt.float16

    nc = bacc.Bacc()
    blob_d = nc.dram_tensor("blob", [K3, WB], f16, kind="ExternalInput")
    out_d = nc.dram_tensor("out", [P, NT, NF], f16, kind="ExternalOutput")

    with TileContext(nc) as tc:
        with (
            tc.tile_pool(name="io", bufs=1) as io,
            tc.tile_pool(name="ps", bufs=1, space="PSUM") as psp,
        ):
            blob = io.tile([K3, WB], f16, tag="blob")
            # split input fetch across three DMA queues, one per ACT chunk,
            # so chunk-0 data is not delayed by later tiles' packets
            c0 = NF + ACT_CH[0] * P
            c1 = c0 + ACT_CH[1] * P
            nc.sync.dma_start(blob[:, :c0], blob_d[:, :c0])
            nc.scalar.dma_start(blob[:, c0:c1], blob_d[:, c0:c1])
            nc.gpsimd.dma_start(blob[:, c1:], blob_d[:, c1:])

            pss, ots = [], []
            t = 0
            for ci, nt_chunk in enumerate(ACT_CH):
                psc = psp.tile([P, nt_chunk, PST], f32, tag=f"mm{ci}")
                otc = io.tile([P, nt_chunk, NF], f16, tag=f"ot{ci}")
                pss.append(psc); ots.append(otc)
                t0 = t
                for k in range(nt_chunk):
                    nc.tensor.matmul(
                        psc[:, k, :NF],
                        blob[:, NF + t * P: NF + (t + 1) * P],
                        blob[:, :NF],
                        start=True, stop=True,
                    )
                    t += 1
                nc.scalar.activation(
                    otc[:], psc[:, :, :NF],
                    mybir.ActivationFunctionType.Exp,
                    scale=float(SCALE),
                )
                eng = nc.scalar if ci == len(ACT_CH) - 1 else nc.sync
                eng.dma_start(out_d[:, t0:t, :], otc[:])
    nc.finalize()
    _NC_CACHE["nc"] = nc
    return nc


def _run_sim(blobs):
    out = []
    for blob in blobs:
        b = blob.astype(np.float32)
        mm = b[:, NF:].T @ b[:, :NF]                 # [NO, NF]
        ex = np.exp(np.minimum(SCALE * mm, 0.0)).astype(np.float16)
        out.append({"out": ex.reshape(NT, P, NF).transpose(1, 0, 2)})
    return out


def kernel(action_logits, target_mean, bbox, scores):
    action_logits = np.asarray(action_logits, np.float32)
    target_mean = np.asarray(target_mean, np.float32)
    bbox = np.asarray(bbox, np.float32)
    scores = np.asarray(scores, np.float32)

    best, hidx_all, geo = _prep(bbox, scores, target_mean, action_logits)

    full = np.zeros((N, N, A), np.float32)
    kernel.last_run = None
    for b0 in range(0, len(hidx_all), BATCH):
        hb = hidx_all[b0:b0 + BATCH]
        k = len(hb)
        blobs = _batch_blobs(hb, k, geo, target_mean)
        if os.environ.get("KERNEL_SIM") == "1":
            results = _run_sim(blobs)
        else:
            from concourse.bass_utils import run_bass_kernel_spmd
            nc = _build_nc()
            kw = {}
            if os.environ.get("KERNEL_TRACE") == "1":
                kw = dict(trace=True, trace_cores=list(range(NCORES)))
            r = run_bass_kernel_spmd(
                nc, [{"blob": b} for b in blobs],
                core_ids=list(range(NCORES)), **kw
            )
            results = r.results
            kernel.last_run = r
        # gather: out[p, t, j*A+a] -> objects x person-local x action
        for c in range(NCORES):
            o = np.asarray(results[c]["out"], np.float32)
            o = o.transpose(1, 0, 2).reshape(NO, HP, A)   # [obj, j, a]
            for j in range(HP):
                hh = b0 + c * HP + j
                if hh >= len(hidx_all):
                    break
                hg = hidx_all[hh]
                lrow = best[hg] * action_logits[hg]       # [A]
                full[hg] = o[:N, j, :] * lrow[None, :]
    return full


# revision 5
# speedup vs baseline: 1.1191x; 1.1191x over previous
import os
import sys

import numpy as np

for _p in ("/opt/trn_rl_repo",):
    if _p not in sys.path and os.path.isdir(_p):
        sys.path.append(_p)

N = 1500          # proposals
A = 64            # action classes
NC_CLS = 81       # detection classes
STD = 0.3
PERSON_IDX = 2
SCALE = 1.0 / (STD * STD)          # exp(SCALE * mm)

NCORES = 8
NO = 1536         # padded object count (12 tiles of 128)
P = 128
NT = NO // P      # 12 object tiles per core
HP = 3            # persons per core
BATCH = NCORES * HP                # 24 persons per device batch
KK = 6 * HP + 1   # 19 logical contraction rows
K3 = 3 * KK       # 57 rows after [Ahi; Alo; Ahi] x [Bhi; Bhi; Blo] stacking
NF = HP * A       # 192 output cols per core (person-local x action)
WB = NF + NO      # blob cols: [B | A]
PST = 256         # psum col stride per tile (2 tiles/bank)
ACT_CH = (4, 4, 4)  # activation chunks, in tiles (bank aligned: 4*256*4B = 2 banks)

TCLAMP = 16.0
LNFLOOR = -20000.0


def _hilo(a):
    hi = a.astype(np.float16)
    lo = (a - hi.astype(np.float32)).astype(np.float16)
    return hi, lo


def _prep(bbox, scores, target_mean, action_logits):
    """Host-side: person selection, box geometry, per-core blobs."""
    best = scores.max(axis=1)
    idx = scores.argmax(axis=1)
    person = idx == PERSON_IDX
    hidx = np.where(person)[0]

    w = bbox[:, 2] - bbox[:, 0]
    h = bbox[:, 3] - bbox[:, 1]
    cx = bbox[:, 0] + 0.5 * w
    cy = bbox[:, 1] + 0.5 * h

    cx_o = np.zeros(NO, np.float32); cx_o[:N] = cx
    cy_o = np.zeros(NO, np.float32); cy_o[:N] = cy
    lw_o = np.zeros(NO, np.float32); lw_o[:N] = np.log(w)
    lh_o = np.zeros(NO, np.float32); lh_o[:N] = np.log(h)
    lnrow = np.full(NO, LNFLOOR, np.float32)
    obj = np.where(person, 0.0, best)
    pos = obj > 0
    lnrow[:N] = np.where(
        pos, np.log(np.maximum(obj, 1e-38)) / SCALE, LNFLOOR
    )
    geo = (cx_o, cy_o, lw_o, lh_o, lnrow, w, h, cx, cy)
    return best, hidx, geo


def _batch_blobs(hb, k, geo, target_mean):
    """Build per-core [K3, WB] fp16 blobs for one batch of <=BATCH persons."""
    cx_o, cy_o, lw_o, lh_o, lnrow, w, h, cx, cy = geo

    invw = np.ones(BATCH, np.float32); invw[:k] = 1.0 / w[hb]
    invh = np.ones(BATCH, np.float32); invh[:k] = 1.0 / h[hb]
    cxh = np.zeros(BATCH, np.float32); cxh[:k] = cx[hb]
    cyh = np.zeros(BATCH, np.float32); cyh[:k] = cy[hb]
    lwh = np.zeros(BATCH, np.float32); lwh[:k] = np.log(w[hb])
    lhh = np.zeros(BATCH, np.float32); lhh[:k] = np.log(h[hb])
    mu = np.zeros((BATCH, A, 4), np.float32); mu[:k] = target_mean[hb]
    m2 = (mu * mu).sum(axis=-1)                      # [BATCH, A]

    tx = np.clip(cx_o[None] * invw[:, None] - (cxh * invw)[:, None],
                 -TCLAMP, TCLAMP)
    ty = np.clip(cy_o[None] * invh[:, None] - (cyh * invh)[:, None],
                 -TCLAMP, TCLAMP)
    tw = np.clip(lw_o[None] - lwh[:, None], -TCLAMP, TCLAMP)
    th = np.clip(lh_o[None] - lhh[:, None], -TCLAMP, TCLAMP)
    e2 = tx * tx + ty * ty + tw * tw + th * th

    # object-side rows T[h, r, o], r in {tx,ty,tw,th, -e2/2, 1}
    T = np.empty((BATCH, 6, NO), np.float32)
    T[:, 0] = tx; T[:, 1] = ty; T[:, 2] = tw; T[:, 3] = th
    T[:, 4] = -0.5 * e2
    T[:, 5] = 1.0

    blobs = []
    for c in range(NCORES):
        a32 = np.empty((KK, NO), np.float32)
        a32[:6 * HP] = T[c * HP:(c + 1) * HP].reshape(6 * HP, NO)
        a32[6 * HP] = lnrow

        b32 = np.zeros((KK, NF), np.float32)
        for j in range(HP):
            hh = c * HP + j
            blk = slice(j * A, (j + 1) * A)
            for cc in range(4):
                b32[6 * j + cc, blk] = mu[hh, :, cc]
            b32[6 * j + 4, blk] = 1.0
            b32[6 * j + 5, blk] = -0.5 * m2[hh]
        b32[6 * HP, :] = 1.0

        ahi, alo = _hilo(a32)
        bhi, blo = _hilo(b32)
        blob = np.empty((K3, WB), np.float16)
        blob[0 * KK:1 * KK, :NF] = bhi
        blob[1 * KK:2 * KK, :NF] = bhi
        blob[2 * KK:3 * KK, :NF] = blo
        blob[0 * KK:1 * KK, NF:] = ahi
        blob[1 * KK:2 * KK, NF:] = alo
        blob[2 * KK:3 * KK, NF:] = ahi
        blobs.append(blob)
    return blobs


_NC_CACHE = {}


def _build_nc():
    if "nc" in _NC_CACHE:
        return _NC_CACHE["nc"]
    import concourse.bacc as bacc
    import concourse.mybir as mybir
    from concourse.tile import TileContext

    f32 = mybir.dt.float32
    f16 = mybir.dt.float16

    nc = bacc.Bacc()
    blob_d = nc.dram_tensor("blob", [K3, WB], f16, kind="ExternalInput")
    out_d = nc.dram_tensor("out", [P, NT, NF], f16, kind="ExternalOutput")

    with TileContext(nc) as tc:
        with (
            tc.tile_pool(name="io", bufs=1) as io,
            tc.tile_pool(name="ps", bufs=1, space="PSUM") as psp,
        ):
            blob = io.tile([K3, WB], f16, tag="blob")
            # sequential fetch on one queue: time-to-first-chunk is floored
            # by per-descriptor HBM read latency, so front-load the bulk
            c0 = NF + (ACT_CH[0] + ACT_CH[1]) * P
            nc.sync.dma_start(blob[:, :c0], blob_d[:, :c0])
            nc.sync.dma_start(blob[:, c0:], blob_d[:, c0:])

            pss, ots = [], []
            t = 0
            for ci, nt_chunk in enumerate(ACT_CH):
                psc = psp.tile([P, nt_chunk, PST], f32, tag=f"mm{ci}")
                otc = io.tile([P, nt_chunk, NF], f16, tag=f"ot{ci}")
                pss.append(psc); ots.append(otc)
                t0 = t
                for k in range(nt_chunk):
                    nc.tensor.matmul(
                        psc[:, k, :NF],
                        blob[:, NF + t * P: NF + (t + 1) * P],
                        blob[:, :NF],
                        start=True, stop=True,
                    )
                    t += 1
                nc.scalar.activation(
                    otc[:], psc[:, :, :NF],
                    mybir.ActivationFunctionType.Exp,
                    scale=float(SCALE),
                )
                eng = nc.scalar if ci == len(ACT_CH) - 1 else nc.sync
                eng.dma_start(out_d[:, t0:t, :], otc[:])
    nc.finalize()
    _NC_CACHE["nc"] = nc
    return nc


def _run_sim(blobs):
    out = []
    for blob in blobs:
        b = blob.astype(np.float32)
        mm = b[:, NF:].T @ b[:, :NF]                 # [NO, NF]
        ex = np.exp(np.minimum(SCALE * mm, 0.0)).astype(np.float16)
        out.append({"out": ex.reshape(NT, P, NF).transpose(1, 0, 2)})
    return out


def kernel(action_logits, target_mean, bbox, scores):
    action_logits = np.asarray(action_logits, np.float32)
    target_mean = np.asarray(target_mean, np.float32)
    bbox = np.asarray(bbox, np.float32)
    scores = np.asarray(scores, np.float32)

    best, hidx_all, geo = _prep(bbox, scores, target_mean, action_logits)

    full = np.zeros((N, N, A), np.float32)
    kernel.last_run = None
    for b0 in range(0, len(hidx_all), BATCH):
        hb = hidx_all[b0:b0 + BATCH]
        k = len(hb)
        blobs = _batch_blobs(hb, k, geo, target_mean)
        if os.environ.get("KERNEL_SIM") == "1":
            results = _run_sim(blobs)
        else:
            from concourse.bass_utils import run_bass_kernel_spmd
            nc = _build_nc()
            kw = {}
            if os.environ.get("KERNEL_TRACE") == "1":
                kw = dict(trace=True, trace_cores=list(range(NCORES)))
            r = run_bass_kernel_spmd(
                nc, [{"blob": b} for b in blobs],
                core_ids=list(range(NCORES)), **kw
            )
            results = r.results
            kernel.last_run = r
        # gather: out[p, t, j*A+a] -> objects x person-local x action
        for c in range(NCORES):
            o = np.asarray(results[c]["out"], np.float32)
            o = o.transpose(1, 0, 2).reshape(NO, HP, A)   # [obj, j, a]
            for j in range(HP):
                hh = b0 + c * HP + j
                if hh >= len(hidx_all):
                    break
                hg = hidx_all[hh]
                lrow = best[hg] * action_logits[hg]       # [A]
                full[hg] = o[:N, j, :] * lrow[None, :]
    return full


# revision 12
# speedup vs baseline: 1.1233x; 1.0037x over previous
import os
import sys

import numpy as np

for _p in ("/opt/trn_rl_repo",):
    if _p not in sys.path and os.path.isdir(_p):
        sys.path.append(_p)

N = 1500          # proposals
A = 64            # action classes
NC_CLS = 81       # detection classes
STD = 0.3
PERSON_IDX = 2
SCALE = 1.0 / (STD * STD)          # exp(SCALE * mm)

NCORES = 8
NO = 1536         # padded object count (12 tiles of 128)
P = 128
NT = NO // P      # 12 object tiles per core
HP = 3            # persons per core
BATCH = NCORES * HP                # 24 persons per device batch
KK = 6 * HP + 1   # 19 logical contraction rows
K3 = 3 * KK       # 57 rows after [Ahi; Alo; Ahi] x [Bhi; Bhi; Blo] stacking
NF = HP * A       # 192 output cols per core (person-local x action)
WB = NF + NO      # blob cols: [B | A]
PST = 256         # psum col stride per tile (2 tiles/bank)
ACT_CH = (4, 4, 4)  # activation chunks, in tiles (bank aligned: 4*256*4B = 2 banks)
# K rows split across both SBUF partition halves: partitions 0..63 are
# served by the 8 even SDMA ports, 64..127 by the 8 odd ports. Splitting
# engages all 16 engines for the (HBM-read-latency-bound) input fetch;
# the matmul runs as two concurrent row-group-packed MMs (tile_position
# derives from base partition 0 / 64).
K1 = 32
K2 = K3 - K1      # 25
PB2 = 32          # partition base of the second K slice

TCLAMP = 16.0
LNFLOOR = -20000.0


def _hilo(a):
    hi = a.astype(np.float16)
    lo = (a - hi.astype(np.float32)).astype(np.float16)
    return hi, lo


def _prep(bbox, scores, target_mean, action_logits):
    """Host-side: person selection, box geometry, per-core blobs."""
    best = scores.max(axis=1)
    idx = scores.argmax(axis=1)
    person = idx == PERSON_IDX
    hidx = np.where(person)[0]

    w = bbox[:, 2] - bbox[:, 0]
    h = bbox[:, 3] - bbox[:, 1]
    cx = bbox[:, 0] + 0.5 * w
    cy = bbox[:, 1] + 0.5 * h

    cx_o = np.zeros(NO, np.float32); cx_o[:N] = cx
    cy_o = np.zeros(NO, np.float32); cy_o[:N] = cy
    lw_o = np.zeros(NO, np.float32); lw_o[:N] = np.log(w)
    lh_o = np.zeros(NO, np.float32); lh_o[:N] = np.log(h)
    lnrow = np.full(NO, LNFLOOR, np.float32)
    obj = np.where(person, 0.0, best)
    pos = obj > 0
    lnrow[:N] = np.where(
        pos, np.log(np.maximum(obj, 1e-38)) / SCALE, LNFLOOR
    )
    geo = (cx_o, cy_o, lw_o, lh_o, lnrow, w, h, cx, cy)
    return best, hidx, geo


def _batch_blobs(hb, k, geo, target_mean):
    """Build per-core [K3, WB] fp16 blobs for one batch of <=BATCH persons."""
    cx_o, cy_o, lw_o, lh_o, lnrow, w, h, cx, cy = geo

    invw = np.ones(BATCH, np.float32); invw[:k] = 1.0 / w[hb]
    invh = np.ones(BATCH, np.float32); invh[:k] = 1.0 / h[hb]
    cxh = np.zeros(BATCH, np.float32); cxh[:k] = cx[hb]
    cyh = np.zeros(BATCH, np.float32); cyh[:k] = cy[hb]
    lwh = np.zeros(BATCH, np.float32); lwh[:k] = np.log(w[hb])
    lhh = np.zeros(BATCH, np.float32); lhh[:k] = np.log(h[hb])
    mu = np.zeros((BATCH, A, 4), np.float32); mu[:k] = target_mean[hb]
    m2 = (mu * mu).sum(axis=-1)                      # [BATCH, A]

    tx = np.clip(cx_o[None] * invw[:, None] - (cxh * invw)[:, None],
                 -TCLAMP, TCLAMP)
    ty = np.clip(cy_o[None] * invh[:, None] - (cyh * invh)[:, None],
                 -TCLAMP, TCLAMP)
    tw = np.clip(lw_o[None] - lwh[:, None], -TCLAMP, TCLAMP)
    th = np.clip(lh_o[None] - lhh[:, None], -TCLAMP, TCLAMP)
    e2 = tx * tx + ty * ty + tw * tw + th * th

    # object-side rows T[h, r, o], r in {tx,ty,tw,th, -e2/2, 1}
    T = np.empty((BATCH, 6, NO), np.float32)
    T[:, 0] = tx; T[:, 1] = ty; T[:, 2] = tw; T[:, 3] = th
    T[:, 4] = -0.5 * e2
    T[:, 5] = 1.0

    blobs = []
    for c in range(NCORES):
        a32 = np.empty((KK, NO), np.float32)
        a32[:6 * HP] = T[c * HP:(c + 1) * HP].reshape(6 * HP, NO)
        a32[6 * HP] = lnrow

        b32 = np.zeros((KK, NF), np.float32)
        for j in range(HP):
            hh = c * HP + j
            blk = slice(j * A, (j + 1) * A)
            for cc in range(4):
                b32[6 * j + cc, blk] = mu[hh, :, cc]
            b32[6 * j + 4, blk] = 1.0
            b32[6 * j + 5, blk] = -0.5 * m2[hh]
        b32[6 * HP, :] = 1.0

        ahi, alo = _hilo(a32)
        bhi, blo = _hilo(b32)
        full = np.empty((K3, WB), np.float16)
        full[0 * KK:1 * KK, :NF] = bhi
        full[1 * KK:2 * KK, :NF] = bhi
        full[2 * KK:3 * KK, :NF] = blo
        full[0 * KK:1 * KK, NF:] = ahi
        full[1 * KK:2 * KK, NF:] = alo
        full[2 * KK:3 * KK, NF:] = ahi
        blobs.append(full)
    return blobs


_NC_CACHE = {}


def _build_nc():
    if "nc" in _NC_CACHE:
        return _NC_CACHE["nc"]
    import concourse.bacc as bacc
    import concourse.mybir as mybir
    from concourse.tile import TileContext

    f32 = mybir.dt.float32
    f16 = mybir.dt.float16

    nc = bacc.Bacc()
    blob_d = nc.dram_tensor("blob", [K3, WB], f16, kind="ExternalInput")
    out_d = nc.dram_tensor("out", [P, NT, NF], f16, kind="ExternalOutput")

    with TileContext(nc) as tc:
        with (
            tc.tile_pool(name="io", bufs=1) as io,
            tc.tile_pool(name="ps", bufs=1, space="PSUM") as psp,
        ):
            blob = io.tile([K3, WB], f16, tag="blob")
            # chunk0+1 sequential on sync (chunk0's descriptors finish
            # first); chunk2 on scalar's queue in parallel
            c0 = NF + ACT_CH[0] * P
            c1 = c0 + ACT_CH[1] * P
            nc.sync.dma_start(blob[:, :c0], blob_d[:, :c0])
            nc.sync.dma_start(blob[:, c0:c1], blob_d[:, c0:c1])
            nc.scalar.dma_start(blob[:, c1:], blob_d[:, c1:])

            pss, ots = [], []
            t = 0
            for ci, nt_chunk in enumerate(ACT_CH):
                psc = psp.tile([P, nt_chunk, PST], f32, tag=f"mm{ci}")
                otc = io.tile([P, nt_chunk, NF], f16, tag=f"ot{ci}")
                pss.append(psc); ots.append(otc)
                t0 = t
                for k in range(nt_chunk):
                    csl = slice(NF + t * P, NF + (t + 1) * P)
                    nc.tensor.matmul(
                        psc[:, k, :NF], blob[:, csl], blob[:, :NF],
                        start=True, stop=True,
                    )
                    t += 1
                nc.scalar.activation(
                    otc[:], psc[:, :, :NF],
                    mybir.ActivationFunctionType.Exp,
                    scale=float(SCALE),
                )
                eng = nc.scalar if ci == len(ACT_CH) - 1 else nc.sync
                eng.dma_start(out_d[:, t0:t, :], otc[:])
    nc.finalize()
    _NC_CACHE["nc"] = nc
    return nc


def _run_sim(blobs):
    out = []
    for blob in blobs:
        b = blob.astype(np.float32)
        mm = b[:, NF:].T @ b[:, :NF]                 # [NO, NF]
        ex = np.exp(np.minimum(SCALE * mm, 0.0)).astype(np.float16)
        out.append({"out": ex.reshape(NT, P, NF).transpose(1, 0, 2)})
    return out


def kernel(action_logits, target_mean, bbox, scores):
    action_logits = np.asarray(action_logits, np.float32)
    target_mean = np.asarray(target_mean, np.float32)
    bbox = np.asarray(bbox, np.float32)
    scores = np.asarray(scores, np.float32)

    best, hidx_all, geo = _prep(bbox, scores, target_mean, action_logits)

    full = np.zeros((N, N, A), np.float32)
    kernel.last_run = None
    for b0 in range(0, len(hidx_all), BATCH):
        hb = hidx_all[b0:b0 + BATCH]
        k = len(hb)
        blobs = _batch_blobs(hb, k, geo, target_mean)
        if os.environ.get("KERNEL_SIM") == "1":
            results = _run_sim(blobs)
        else:
            from concourse.bass_utils import run_bass_kernel_spmd
            nc = _build_nc()
            kw = {}
            if os.environ.get("KERNEL_TRACE") == "1":
                kw = dict(trace=True, trace_cores=list(range(NCORES)))
            r = run_bass_kernel_spmd(
                nc, [{"blob": b} for b in blobs],
                core_ids=list(range(NCORES)), **kw
            )
            results = r.results
            kernel.last_run = r
        # gather: out[p, t, j*A+a] -> objects x person-local x action
        for c in range(NCORES):
            o = np.asarray(results[c]["out"], np.float32)
            o = o.transpose(1, 0, 2).reshape(NO, HP, A)   # [obj, j, a]
            for j in range(HP):
                hh = b0 + c * HP + j
                if hh >= len(hidx_all):
                    break
                hg = hidx_all[hh]
                lrow = best[hg] * action_logits[hg]       # [A]
                full[hg] = o[:N, j, :] * lrow[None, :]
    return full
